# revision 51
# baseline (speedup 1.0000x reference)
"""AttentionWithRoPE distributed Trainium2 kernel (8 NeuronCores).

Sharding: pure 8-way tensor parallel over heads (2 heads = 128 hidden cols
per core), both batches on every core (seq concatenated to 4096 cols).
Everything stays transposed ([feature, seq] layouts) so no on-device
transposes are needed anywhere:
  - QKV projections consume xT (host-transposed, batch-concat, bf16) as the
    moving operand, streamed in [128, 512] chunks.
  - RoPE applied on qT/kT [d, s] tiles; the 32-row half-rotation is done with
    sbuf->sbuf DMAs (DVE ops must keep operand start-partitions equal), with
    a sign-folded sin table so no negation op is needed.
  - scores^T = kT.T @ qT per (head, batch) in [ks, qs] layout, exp on ScalarE
    (no max subtraction needed: scores are ~N(0,1) here).
  - ctx^T via matmul with a ones-column appended to V (65th column gives the
    softmax denominator for free), normalized via GpSimd partition-broadcast
    of 1/rowsum + one DVE multiply (fuses psum->sbuf copy + cast to bf16).
  - AllToAll (bf16, all 8 cores) exchanges 512-row blocks of ctx^T; received
    slabs are exactly the o-chunks the output projection consumes.
  - Output projection with full Wo produces out^T [1024, 512] for this
    core's 512 global rows; host transposes back (free).
Bias folds (host side): v-bias folds into the output bias exactly (softmax
rows sum to 1); q is pre-scaled by 1/sqrt(64) inside its bias-copy.
Compute dtype bf16 (fp32 PSUM accumulation); all weights/activations are
converted to bf16 on the host.
"""

import numpy as np

HID = 1024
S = 2048
SB = 2 * S       # both batches, seq-concatenated
NHEAD = 16
D = 64
HPC = 2          # heads per core
OSL = 128        # hidden slice per core (HPC * D)
RB = 512         # global row block per core after AllToAll
NC = 8
ROPE_BASE = 10000.0

_cached = None
_last_in_maps = None


def _build_nc():
    import concourse.bacc as bacc
    import concourse.mybir as mybir
    from concourse import tile

    f32 = mybir.dt.float32
    bf16 = mybir.dt.bfloat16
    AF = mybir.ActivationFunctionType

    nc = bacc.Bacc(None, target_bir_lowering=False)

    xT = nc.declare_dram_parameter("xT", [HID, SB], bf16, isOutput=False)
    wqT = nc.declare_dram_parameter("wqT", [HID, OSL], bf16, isOutput=False)
    wkT = nc.declare_dram_parameter("wkT", [HID, OSL], bf16, isOutput=False)
    wvT = nc.declare_dram_parameter("wvT", [HID, OSL], bf16, isOutput=False)
    woT = nc.declare_dram_parameter("woT", [HID, HID], bf16, isOutput=False)
    bqd = nc.declare_dram_parameter("bq", [128, 1], f32, isOutput=False)
    bkd = nc.declare_dram_parameter("bk", [128, 1], f32, isOutput=False)
    bod = nc.declare_dram_parameter("bo2", [128, 8], f32, isOutput=False)
    cosd = nc.declare_dram_parameter("cosT", [128, SB], bf16, isOutput=False)
    sind = nc.declare_dram_parameter("sinS", [128, SB], bf16, isOutput=False)
    out_ext = nc.declare_dram_parameter("out", [HID, RB], bf16, isOutput=True)

    a2a_in = nc.dram_tensor("a2a_in", [NC, OSL, RB], bf16)
    a2a_out = nc.dram_tensor("a2a_out", [NC, OSL, RB], bf16)

    NHC = HID // 128  # 8 hidden chunks
    NSG = SB // 512   # 8 seq groups

    from concourse.tile_rust import add_dep_helper

    with tile.TileContext(nc) as tc:
        with (
            tc.tile_pool(name="persist", bufs=1) as pp,
            tc.tile_pool(name="xs", bufs=18) as xp,
            tc.tile_pool(name="work", bufs=2) as wp,
            tc.tile_pool(name="exp", bufs=2) as ep,
        ):
            # ---------- consts ----------
            def pload(dram_ap, shape, dt_, tag):
                t = pp.tile(shape, dt_, tag=tag, name=tag)
                nc.sync.dma_start(out=t[:, :], in_=dram_ap)
                return t

            wqb = [pload(wqT[128 * c:128 * (c + 1), :], [128, OSL], bf16,
                         f"wqb{c}") for c in range(NHC)]
            wkb = [pload(wkT[128 * c:128 * (c + 1), :], [128, OSL], bf16,
                         f"wkb{c}") for c in range(NHC)]
            wvb = [pload(wvT[128 * c:128 * (c + 1), :], [128, OSL], bf16,
                         f"wvb{c}") for c in range(NHC)]
            bq_sb = pload(bqd[:, :], [128, 1], f32, "bq")
            bk_sb = pload(bkd[:, :], [128, 1], f32, "bk")
            bo_sb = pload(bod[:, :], [128, 8], f32, "bo")
            # cos/sin are not needed until the first rope quarter (after
            # seq-group 1): load them in column halves so the big transfers
            # don't delay the first projection matmuls.
            cos_sb = pp.tile([128, SB], bf16, tag="cos")
            sin_sb = pp.tile([128, SB], bf16, tag="sin")

            # PSUM pools for phases 1-4 (8 banks exactly); closed before the
            # output projection which needs all 8 banks for its accumulators.
            # Tags are shared across phases: "mm512" serves qk-proj psums then
            # the score tiles; "acc" serves v-proj psums then ctx accumulators.
            _cmA = tc.tile_pool(name="psA", bufs=2, space="PSUM")
            _cmB = tc.tile_pool(name="psB", bufs=4, space="PSUM")
            psA = _cmA.__enter__()
            psB = _cmB.__enter__()

            # ---------- phase 1: QKV projections (x streamed) + fused RoPE ----
            # RoPE for seq-quarter i runs as soon as seq-groups 2i, 2i+1 have
            # projected, overlapping the remaining projection matmuls.
            qsb = wp.tile([128, SB], bf16, tag="qsb", bufs=1)
            ksb = wp.tile([128, SB], bf16, tag="ksb", bufs=1)
            # Per-head rope outputs, d zero-padded to 128 partitions so K=128
            # score matmuls keep the full PE array active (HAM stays warm).
            qrh = [pp.tile([128, SB], bf16, tag=f"qrh{h}", name=f"qrh{h}")
                   for h in range(HPC)]
            krh = [pp.tile([128, SB], bf16, tag=f"krh{h}", name=f"krh{h}")
                   for h in range(HPC)]
            for t in qrh + krh:
                nc.gpsimd.memset(t[64:128, :], 0.0)

            def rope_quarter(src, dsts, q4):
                sl = slice(1024 * q4, 1024 * (q4 + 1))
                qswp = wp.tile([128, 1024], bf16, tag="qswp")
                for blk in range(4):
                    dlo = 32 * blk
                    srow = 32 * (blk + 1) if blk % 2 == 0 else 32 * (blk - 1)
                    nc.sync.dma_start(
                        out=qswp[dlo:dlo + 32, :],
                        in_=src[srow:srow + 32, sl])
                t1 = wp.tile([128, 1024], f32, tag="ropet1")
                t2 = wp.tile([128, 1024], f32, tag="ropet2")
                rt = wp.tile([128, 1024], bf16, tag="ropert")
                nc.vector.tensor_mul(t1[:, :], src[:, sl], cos_sb[:, sl])
                nc.vector.tensor_mul(t2[:, :], qswp[:, :], sin_sb[:, sl])
                nc.vector.tensor_add(rt[:, :], t1[:, :], t2[:, :])
                for h in range(HPC):
                    nc.sync.dma_start(
                        out=dsts[h][0:64, sl],
                        in_=rt[64 * h:64 * (h + 1), :])

            vsb = []
            qkps = {}
            _gate0 = None
            _gate1 = None
            for sg in range(NSG):
                xbt = []
                for c in range(NHC):
                    xb = xp.tile([128, 512], bf16, tag="xb")
                    nc.sync.dma_start(
                        out=xb[:, :],
                        in_=xT[128 * c:128 * (c + 1), 512 * sg:512 * (sg + 1)])
                    xbt.append(xb)
                if sg == 1:
                    # defer the big cos/sin transfers behind the first matmul
                    # so they don't steal DMA bandwidth from the critical
                    # wq/x loads (descriptor-level fair sharing would
                    # otherwise delay the first matmul by ~15us)
                    for qrt in range(4):
                        hs = slice(1024 * qrt, 1024 * (qrt + 1))
                        d1 = nc.sync.dma_start(out=cos_sb[:, hs],
                                               in_=cosd[:, hs])
                        d2 = nc.sync.dma_start(out=sin_sb[:, hs],
                                               in_=sind[:, hs])
                        if _gate0 is not None:
                            add_dep_helper(d1.ins, _gate0.ins,
                                           reason="defer cos load")
                            add_dep_helper(d2.ins, _gate0.ins,
                                           reason="defer sin load")
                half = sg % 2
                for key, wb, bias, scale, dest in (
                    ("q", wqb, bq_sb, 0.125, qsb),
                    ("k", wkb, bk_sb, 1.0, ksb),
                ):
                    if half == 0:
                        qkps[key] = psA.tile([128, 1024], f32, tag="mm1024",
                                             name=f"qkps{key}")
                    ps = qkps[key]
                    for c in range(NHC):
                        mm = nc.tensor.matmul(
                            ps[:, 512 * half:512 * (half + 1)],
                            lhsT=wb[c][:, :], rhs=xbt[c][:, :],
                            start=(c == 0), stop=(c == NHC - 1))
                        if _gate0 is None:
                            _gate0 = mm
                        if sg == 4 and _gate1 is None:
                            _gate1 = mm
                    if half == 1:
                        nc.vector.tensor_scalar(
                            dest[:, 1024 * (sg // 2):1024 * (sg // 2 + 1)],
                            ps[:, :], scale, bias[:, 0:1],
                            mybir.AluOpType.mult, mybir.AluOpType.add)
                for st4 in range(4):
                    st = 4 * sg + st4
                    ps = psB.tile([128, OSL], f32, tag="acc",
                                  padded_shape=[128, 512])
                    for c in range(NHC):
                        nc.tensor.matmul(
                            ps[:, :],
                            lhsT=xbt[c][:, 128 * st4:128 * (st4 + 1)],
                            rhs=wvb[c][:, :],
                            start=(c == 0), stop=(c == NHC - 1))
                    # Per head: [v(64) | ones(1) | zeros(63)] -> M=128 ctx
                    # matmuls keep the full PE array active (HAM stays warm).
                    vt = pp.tile([128, 2 * 128], bf16,
                                 tag=f"vsb{st}", name=f"vsb{st}")
                    nc.gpsimd.memset(vt[:, :], 0.0)
                    for h in range(HPC):
                        nc.gpsimd.memset(vt[:, 128 * h + 64:128 * h + 65], 1.0)
                        nc.vector.tensor_copy(
                            vt[:, 128 * h:128 * h + 64],
                            ps[:, 64 * h:64 * (h + 1)])
                    vsb.append(vt)
                if sg % 2 == 1:
                    rope_quarter(qsb, qrh, sg // 2)
                    rope_quarter(ksb, krh, sg // 2)

            # ---------- phase 3: attention, both heads packed per pass ----------
            # The two heads' K=64 score matmuls are issued back-to-back with
            # lhsT/rhs at base partitions 0 and 64, which auto-derives PE
            # tile_position (0,0)/(64,0): both run concurrently in the array.
            # Per-head ctx tiles so every DVE op starts at partition 0.
            ctxh = [pp.tile([64, SB], bf16, tag=f"ctx{h}", name=f"ctx{h}")
                    for h in range(HPC)]
            for b in range(2):
                for qs in range(4):
                    q0 = S * b + 512 * qs
                    cpsA = psB.tile([128, 512], f32, tag="acc")
                    cpsB = psB.tile([128, 512], f32, tag="acc")
                    for ks in range(16):
                        k0 = S * b + 128 * ks
                        # one 2-bank psum tile holds both heads' score tiles,
                        # so a single wide EXP serves both ctx matmuls
                        sps = psA.tile([128, 1024], f32, tag="mm1024")
                        nc.tensor.matmul(
                            sps[:, 0:512], lhsT=krh[0][:, k0:k0 + 128],
                            rhs=qrh[0][:, q0:q0 + 512],
                            start=True, stop=True)
                        nc.tensor.matmul(
                            sps[:, 512:1024], lhsT=krh[1][:, k0:k0 + 128],
                            rhs=qrh[1][:, q0:q0 + 512],
                            start=True, stop=True)
                        et = ep.tile([128, 1024], bf16, tag="expT", bufs=4)
                        nc.scalar.activation(et[:, :], sps[:, :], AF.Exp)
                        vt = vsb[(S * b) // 128 + ks]
                        nc.tensor.matmul(
                            cpsA[:, :], lhsT=vt[:, 0:128], rhs=et[:, 0:512],
                            start=(ks == 0), stop=(ks == 15))
                        nc.tensor.matmul(
                            cpsB[:, :], lhsT=vt[:, 128:256],
                            rhs=et[:, 512:1024],
                            start=(ks == 0), stop=(ks == 15))
                    for h, cps in ((0, cpsA), (1, cpsB)):
                        # rowsum lives on psum partition 64; hop it to sbuf,
                        # DMA-reshape to [128,4] so reciprocal runs 128 lanes
                        # wide, reshape back, broadcast, normalize.
                        rs65 = ep.tile([65, 512], f32, tag="rec65")
                        nc.vector.tensor_copy(rs65[64:65, :], cps[64:65, :])
                        rsP = ep.tile([128, 4], f32, tag="rsP")
                        nc.sync.dma_start(out=rsP[:, :], in_=rs65[64:65, :])
                        rPr = ep.tile([128, 4], f32, tag="rPr")
                        nc.vector.reciprocal(rPr[:, :], rsP[:, :])
                        rec0 = ep.tile([1, 512], f32, tag="rec0")
                        nc.sync.dma_start(out=rec0[:, :], in_=rPr[:, :])
                        rb = ep.tile([64, 512], f32, tag="recb")
                        nc.gpsimd.partition_broadcast(rb[:, :], rec0[:, :])
                        nc.vector.tensor_mul(
                            ctxh[h][:, q0:q0 + 512], cps[0:64, :], rb[:, :])
                        # this (b,qs) round is destination-slab 4b+qs: ship it
                        # to the a2a buffer as soon as it is normalized
                        nc.sync.dma_start(
                            out=a2a_in[4 * b + qs, 64 * h:64 * (h + 1), :],
                            in_=ctxh[h][:, q0:q0 + 512])

            # ---------- phase 4: AllToAll ----------
            nc.gpsimd.collective_compute(
                "AllToAll", mybir.AluOpType.bypass,
                replica_groups=[list(range(NC))],
                ins=[a2a_in.ap().opt()],
                outs=[a2a_out.ap().opt()])

            # ---------- phase 5: output projection ----------
            _cmB.__exit__(None, None, None)
            _cmA.__exit__(None, None, None)
            _cmO = tc.tile_pool(name="psO", bufs=1, space="PSUM")
            psO = _cmO.__enter__()

            wob = []
            for c in range(NHC):
                t = pp.tile([128, HID], bf16, tag=f"wob{c}", name=f"wob{c}")
                d = nc.sync.dma_start(out=t[:, :],
                                      in_=woT[128 * c:128 * (c + 1), :])
                if _gate1 is not None:
                    add_dep_helper(d.ins, _gate1.ins, reason="defer wob load")
                wob.append(t)

            # Keep the PE array (HAM) warm across the AllToAll wait: a chain
            # of full-array matmuls anchored on the last ctx tile (via the
            # dumsrc copy) so they cannot run before attention finishes.
            # Result is consumed by a dead-store DMA so DCE keeps the chain.
            dumsrc = pp.tile([128, 512], bf16, tag="dumsrc")
            nc.gpsimd.memset(dumsrc[:, :], 0.0)
            nc.vector.tensor_copy(
                dumsrc[0:64, :], ctxh[1][:, SB - 512:SB])
            dum = psO.tile([128, 512], f32, tag="dum", bufs=1)
            for i in range(230):
                nc.tensor.matmul(
                    dum[:, :], lhsT=wob[0][:, 0:128], rhs=dumsrc[:, :],
                    start=True, stop=True)
            dumr = ep.tile([128, 512], f32, tag="dumr")
            nc.vector.tensor_copy(dumr[:, :], dum[:, :])
            dead = nc.dram_tensor("dead", [128, 512], f32)
            nc.sync.dma_start(out=dead[:, :], in_=dumr[:, :])
            # Load all 8 received o-chunks first (1MB total), then run the
            # accumulation ot-outer so each out-tile finishes early and its
            # bias-add + store overlap the remaining matmuls.
            cxs = []
            for c in range(NHC):
                cx = pp.tile([128, RB], bf16, tag=f"cxb{c}", name=f"cxb{c}")
                nc.sync.dma_start(out=cx[:, :], in_=a2a_out[c, :, :])
                cxs.append(cx)
            for ot in range(8):
                ops = psO.tile([128, 512], f32, tag="ops", bufs=4)
                for c in range(NHC):
                    nc.tensor.matmul(
                        ops[:, :],
                        lhsT=wob[c][:, 128 * ot:128 * (ot + 1)],
                        rhs=cxs[c][:, :],
                        start=(c == 0), stop=(c == NHC - 1))
                osb = ep.tile([128, RB], bf16, tag="osb", bufs=3)
                nc.scalar.activation(
                    osb[:, :], ops[:, :], AF.Identity,
                    bias=bo_sb[:, ot:ot + 1], scale=1.0)
                nc.sync.dma_start(
                    out=out_ext[128 * ot:128 * (ot + 1), :], in_=osb[:, :])
            _cmO.__exit__(None, None, None)

    nc.finalize()
    return nc


def _host_tables():
    inv = 1.0 / (ROPE_BASE ** (np.arange(0, D, 2, dtype=np.float64) / D))
    pos = np.arange(S, dtype=np.float64)
    freqs = np.outer(pos, inv)                      # [S, 32]
    emb = np.concatenate([freqs, freqs], axis=-1)   # [S, 64]
    cosT = np.cos(emb).T.astype(np.float32)         # [64, S]
    sinT = np.sin(emb).T.astype(np.float32)
    sinS = np.concatenate([-sinT[:32], sinT[32:]], axis=0)
    cos2 = np.ascontiguousarray(np.tile(cosT, (2, 2)))   # [128, 2S]
    sin2 = np.ascontiguousarray(np.tile(sinS, (2, 2)))
    return cos2, sin2


def kernel(**inputs):
    import ml_dtypes
    from concourse.bass_utils import run_bass_kernel_spmd

    global _cached, _last_in_maps
    if _cached is None:
        _cached = _build_nc()
    nc = _cached

    bf = ml_dtypes.bfloat16
    hs = np.asarray(inputs["hidden_states"], dtype=np.float32)
    Wq = np.asarray(inputs["Wq"], dtype=np.float32)
    bq = np.asarray(inputs["bq"], dtype=np.float32)
    Wk = np.asarray(inputs["Wk"], dtype=np.float32)
    bk = np.asarray(inputs["bk"], dtype=np.float32)
    Wv = np.asarray(inputs["Wv"], dtype=np.float32)
    bv = np.asarray(inputs["bv"], dtype=np.float32)
    Wo = np.asarray(inputs["Wo"], dtype=np.float32)
    bo = np.asarray(inputs["bo"], dtype=np.float32)

    cos2, sin2 = _host_tables()
    cos2 = cos2.astype(bf)
    sin2 = sin2.astype(bf)
    bo2 = bo + bv @ Wo.T                                 # fold v-bias exactly
    bo2m = np.ascontiguousarray(bo2.reshape(8, 128).T)   # [128, 8]
    xTfull = np.ascontiguousarray(
        np.concatenate([hs[0].T, hs[1].T], axis=1)).astype(bf)  # [1024, 4096]
    woTc = np.ascontiguousarray(Wo.T).astype(bf)

    in_maps = []
    for c in range(NC):
        sl = slice(OSL * c, OSL * (c + 1))
        in_maps.append({
            "xT": xTfull,
            "wqT": np.ascontiguousarray(Wq[sl, :].T).astype(bf),
            "wkT": np.ascontiguousarray(Wk[sl, :].T).astype(bf),
            "wvT": np.ascontiguousarray(Wv[sl, :].T).astype(bf),
            "woT": woTc,
            "bq": np.ascontiguousarray((bq[sl] * 0.125)[:, None]),
            "bk": np.ascontiguousarray(bk[sl][:, None]),
            "bo2": bo2m,
            "cosT": cos2,
            "sinS": sin2,
        })

    _last_in_maps = in_maps
    res = run_bass_kernel_spmd(nc, in_maps, core_ids=list(range(NC)))
    out = np.empty((2, S, HID), dtype=np.float32)
    for c in range(NC):
        b, g = divmod(c, 4)
        out[b, RB * g:RB * (g + 1), :] = res.results[c]["out"].T.astype(np.float32)
    return out


# revision 52
# speedup vs baseline: 1.0056x; 1.0056x over previous
"""AttentionWithRoPE distributed Trainium2 kernel (8 NeuronCores).

Sharding: pure 8-way tensor parallel over heads (2 heads = 128 hidden cols
per core), both batches on every core (seq concatenated to 4096 cols).
Everything stays transposed ([feature, seq] layouts) so no on-device
transposes are needed anywhere:
  - QKV projections consume xT (host-transposed, batch-concat, bf16) as the
    moving operand, streamed in [128, 512] chunks.
  - RoPE applied on qT/kT [d, s] tiles; the 32-row half-rotation is done with
    sbuf->sbuf DMAs (DVE ops must keep operand start-partitions equal), with
    a sign-folded sin table so no negation op is needed.
  - scores^T = kT.T @ qT per (head, batch) in [ks, qs] layout, exp on ScalarE
    (no max subtraction needed: scores are ~N(0,1) here).
  - ctx^T via matmul with a ones-column appended to V (65th column gives the
    softmax denominator for free), normalized via GpSimd partition-broadcast
    of 1/rowsum + one DVE multiply (fuses psum->sbuf copy + cast to bf16).
  - AllToAll (bf16, all 8 cores) exchanges 512-row blocks of ctx^T; received
    slabs are exactly the o-chunks the output projection consumes.
  - Output projection with full Wo produces out^T [1024, 512] for this
    core's 512 global rows; host transposes back (free).
Bias folds (host side): v-bias folds into the output bias exactly (softmax
rows sum to 1); q is pre-scaled by 1/sqrt(64) inside its bias-copy.
Compute dtype bf16 (fp32 PSUM accumulation); all weights/activations are
converted to bf16 on the host.
"""

import numpy as np

HID = 1024
S = 2048
SB = 2 * S       # both batches, seq-concatenated
NHEAD = 16
D = 64
HPC = 2          # heads per core
OSL = 128        # hidden slice per core (HPC * D)
RB = 512         # global row block per core after AllToAll
NC = 8
ROPE_BASE = 10000.0

_cached = None
_last_in_maps = None


def _build_nc():
    import concourse.bacc as bacc
    import concourse.mybir as mybir
    from concourse import tile

    f32 = mybir.dt.float32
    bf16 = mybir.dt.bfloat16
    AF = mybir.ActivationFunctionType

    nc = bacc.Bacc(None, target_bir_lowering=False)

    xT = nc.declare_dram_parameter("xT", [HID, SB], bf16, isOutput=False)
    wqT = nc.declare_dram_parameter("wqT", [HID, OSL], bf16, isOutput=False)
    wkT = nc.declare_dram_parameter("wkT", [HID, OSL], bf16, isOutput=False)
    wvT = nc.declare_dram_parameter("wvT", [HID, OSL], bf16, isOutput=False)
    woT = nc.declare_dram_parameter("woT", [HID, HID], bf16, isOutput=False)
    bqd = nc.declare_dram_parameter("bq", [128, 1], f32, isOutput=False)
    bkd = nc.declare_dram_parameter("bk", [128, 1], f32, isOutput=False)
    bod = nc.declare_dram_parameter("bo2", [128, 8], f32, isOutput=False)
    cosd = nc.declare_dram_parameter("cosT", [128, SB], bf16, isOutput=False)
    sind = nc.declare_dram_parameter("sinS", [128, SB], bf16, isOutput=False)
    out_ext = nc.declare_dram_parameter("out", [HID, RB], bf16, isOutput=True)

    a2a_in = nc.dram_tensor("a2a_in", [NC, OSL, RB], bf16)
    a2a_out = nc.dram_tensor("a2a_out", [NC, OSL, RB], bf16)

    NHC = HID // 128  # 8 hidden chunks
    NSG = SB // 512   # 8 seq groups

    from concourse.tile_rust import add_dep_helper

    with tile.TileContext(nc) as tc:
        with (
            tc.tile_pool(name="persist", bufs=1) as pp,
            tc.tile_pool(name="xs", bufs=18) as xp,
            tc.tile_pool(name="work", bufs=2) as wp,
            tc.tile_pool(name="exp", bufs=2) as ep,
        ):
            # ---------- consts ----------
            def pload(dram_ap, shape, dt_, tag):
                t = pp.tile(shape, dt_, tag=tag, name=tag)
                nc.sync.dma_start(out=t[:, :], in_=dram_ap)
                return t

            wqb = [pload(wqT[128 * c:128 * (c + 1), :], [128, OSL], bf16,
                         f"wqb{c}") for c in range(NHC)]
            wkb = [pload(wkT[128 * c:128 * (c + 1), :], [128, OSL], bf16,
                         f"wkb{c}") for c in range(NHC)]
            wvb = [pload(wvT[128 * c:128 * (c + 1), :], [128, OSL], bf16,
                         f"wvb{c}") for c in range(NHC)]
            bq_sb = pload(bqd[:, :], [128, 1], f32, "bq")
            bk_sb = pload(bkd[:, :], [128, 1], f32, "bk")
            bo_sb = pload(bod[:, :], [128, 8], f32, "bo")
            # cos/sin are not needed until the first rope quarter (after
            # seq-group 1): load them in column halves so the big transfers
            # don't delay the first projection matmuls.
            cos_sb = pp.tile([128, SB], bf16, tag="cos")
            sin_sb = pp.tile([128, SB], bf16, tag="sin")

            # PSUM pools for phases 1-4 (8 banks exactly); closed before the
            # output projection which needs all 8 banks for its accumulators.
            # Tags are shared across phases: "mm512" serves qk-proj psums then
            # the score tiles; "acc" serves v-proj psums then ctx accumulators.
            _cmA = tc.tile_pool(name="psA", bufs=2, space="PSUM")
            _cmB = tc.tile_pool(name="psB", bufs=4, space="PSUM")
            psA = _cmA.__enter__()
            psB = _cmB.__enter__()

            # ---------- phase 1: QKV projections (x streamed) + fused RoPE ----
            # RoPE for seq-quarter i runs as soon as seq-groups 2i, 2i+1 have
            # projected, overlapping the remaining projection matmuls.
            qsb = wp.tile([128, SB], bf16, tag="qsb", bufs=1)
            ksb = wp.tile([128, SB], bf16, tag="ksb", bufs=1)
            # Per-head rope outputs, d zero-padded to 128 partitions so K=128
            # score matmuls keep the full PE array active (HAM stays warm).
            qrh = [pp.tile([128, SB], bf16, tag=f"qrh{h}", name=f"qrh{h}")
                   for h in range(HPC)]
            krh = [pp.tile([128, SB], bf16, tag=f"krh{h}", name=f"krh{h}")
                   for h in range(HPC)]
            for t in qrh + krh:
                nc.gpsimd.memset(t[64:128, :], 0.0)

            def rope_quarter(src, dsts, q4):
                sl = slice(1024 * q4, 1024 * (q4 + 1))
                qswp = wp.tile([128, 1024], bf16, tag="qswp")
                for blk in range(4):
                    dlo = 32 * blk
                    srow = 32 * (blk + 1) if blk % 2 == 0 else 32 * (blk - 1)
                    nc.sync.dma_start(
                        out=qswp[dlo:dlo + 32, :],
                        in_=src[srow:srow + 32, sl])
                t1 = wp.tile([128, 1024], f32, tag="ropet1")
                t2 = wp.tile([128, 1024], f32, tag="ropet2")
                rt = wp.tile([128, 1024], bf16, tag="ropert")
                nc.vector.tensor_mul(t1[:, :], src[:, sl], cos_sb[:, sl])
                nc.vector.tensor_mul(t2[:, :], qswp[:, :], sin_sb[:, sl])
                nc.vector.tensor_add(rt[:, :], t1[:, :], t2[:, :])
                for h in range(HPC):
                    nc.sync.dma_start(
                        out=dsts[h][0:64, sl],
                        in_=rt[64 * h:64 * (h + 1), :])

            vsb = []
            qkps = {}
            _gate0 = None
            _gate1 = None
            for sg in range(NSG):
                xbt = []
                for c in range(NHC):
                    xb = xp.tile([128, 512], bf16, tag="xb")
                    nc.sync.dma_start(
                        out=xb[:, :],
                        in_=xT[128 * c:128 * (c + 1), 512 * sg:512 * (sg + 1)])
                    xbt.append(xb)
                if sg == 1:
                    # defer the big cos/sin transfers behind the first matmul
                    # so they don't steal DMA bandwidth from the critical
                    # wq/x loads (descriptor-level fair sharing would
                    # otherwise delay the first matmul by ~15us)
                    for qrt in range(4):
                        hs = slice(1024 * qrt, 1024 * (qrt + 1))
                        d1 = nc.sync.dma_start(out=cos_sb[:, hs],
                                               in_=cosd[:, hs])
                        d2 = nc.sync.dma_start(out=sin_sb[:, hs],
                                               in_=sind[:, hs])
                        del d1, d2
                half = sg % 2
                for key, wb, bias, scale, dest in (
                    ("q", wqb, bq_sb, 0.125, qsb),
                    ("k", wkb, bk_sb, 1.0, ksb),
                ):
                    if half == 0:
                        qkps[key] = psA.tile([128, 1024], f32, tag="mm1024",
                                             name=f"qkps{key}")
                    ps = qkps[key]
                    for c in range(NHC):
                        mm = nc.tensor.matmul(
                            ps[:, 512 * half:512 * (half + 1)],
                            lhsT=wb[c][:, :], rhs=xbt[c][:, :],
                            start=(c == 0), stop=(c == NHC - 1))
                        if _gate0 is None:
                            _gate0 = mm
                        if sg == 4 and _gate1 is None:
                            _gate1 = mm
                    if half == 1:
                        nc.vector.tensor_scalar(
                            dest[:, 1024 * (sg // 2):1024 * (sg // 2 + 1)],
                            ps[:, :], scale, bias[:, 0:1],
                            mybir.AluOpType.mult, mybir.AluOpType.add)
                for st4 in range(4):
                    st = 4 * sg + st4
                    ps = psB.tile([128, OSL], f32, tag="acc",
                                  padded_shape=[128, 512])
                    for c in range(NHC):
                        nc.tensor.matmul(
                            ps[:, :],
                            lhsT=xbt[c][:, 128 * st4:128 * (st4 + 1)],
                            rhs=wvb[c][:, :],
                            start=(c == 0), stop=(c == NHC - 1))
                    # Per head: [v(64) | ones(1) | zeros(63)] -> M=128 ctx
                    # matmuls keep the full PE array active (HAM stays warm).
                    vt = pp.tile([128, 2 * 128], bf16,
                                 tag=f"vsb{st}", name=f"vsb{st}")
                    nc.gpsimd.memset(vt[:, :], 0.0)
                    for h in range(HPC):
                        nc.gpsimd.memset(vt[:, 128 * h + 64:128 * h + 65], 1.0)
                        nc.vector.tensor_copy(
                            vt[:, 128 * h:128 * h + 64],
                            ps[:, 64 * h:64 * (h + 1)])
                    vsb.append(vt)
                if sg % 2 == 1:
                    rope_quarter(qsb, qrh, sg // 2)
                    rope_quarter(ksb, krh, sg // 2)

            # ---------- phase 3: attention, both heads packed per pass ----------
            # The two heads' K=64 score matmuls are issued back-to-back with
            # lhsT/rhs at base partitions 0 and 64, which auto-derives PE
            # tile_position (0,0)/(64,0): both run concurrently in the array.
            # Per-head ctx tiles so every DVE op starts at partition 0.
            ctxh = [pp.tile([64, SB], bf16, tag=f"ctx{h}", name=f"ctx{h}")
                    for h in range(HPC)]
            for b in range(2):
                for qs in range(4):
                    q0 = S * b + 512 * qs
                    cpsA = psB.tile([128, 512], f32, tag="acc")
                    cpsB = psB.tile([128, 512], f32, tag="acc")
                    for ks in range(16):
                        k0 = S * b + 128 * ks
                        # one 2-bank psum tile holds both heads' score tiles,
                        # so a single wide EXP serves both ctx matmuls
                        sps = psA.tile([128, 1024], f32, tag="mm1024")
                        nc.tensor.matmul(
                            sps[:, 0:512], lhsT=krh[0][:, k0:k0 + 128],
                            rhs=qrh[0][:, q0:q0 + 512],
                            start=True, stop=True)
                        nc.tensor.matmul(
                            sps[:, 512:1024], lhsT=krh[1][:, k0:k0 + 128],
                            rhs=qrh[1][:, q0:q0 + 512],
                            start=True, stop=True)
                        et = ep.tile([128, 1024], bf16, tag="expT", bufs=4)
                        nc.scalar.activation(et[:, :], sps[:, :], AF.Exp)
                        vt = vsb[(S * b) // 128 + ks]
                        nc.tensor.matmul(
                            cpsA[:, :], lhsT=vt[:, 0:128], rhs=et[:, 0:512],
                            start=(ks == 0), stop=(ks == 15))
                        nc.tensor.matmul(
                            cpsB[:, :], lhsT=vt[:, 128:256],
                            rhs=et[:, 512:1024],
                            start=(ks == 0), stop=(ks == 15))
                    for h, cps in ((0, cpsA), (1, cpsB)):
                        # rowsum lives on psum partition 64; hop it to sbuf,
                        # DMA-reshape to [128,4] so reciprocal runs 128 lanes
                        # wide, reshape back, broadcast, normalize.
                        rs65 = ep.tile([65, 512], f32, tag="rec65")
                        nc.vector.tensor_copy(rs65[64:65, :], cps[64:65, :])
                        rsP = ep.tile([128, 4], f32, tag="rsP")
                        nc.sync.dma_start(out=rsP[:, :], in_=rs65[64:65, :])
                        rPr = ep.tile([128, 4], f32, tag="rPr")
                        nc.vector.reciprocal(rPr[:, :], rsP[:, :])
                        rec0 = ep.tile([1, 512], f32, tag="rec0")
                        nc.sync.dma_start(out=rec0[:, :], in_=rPr[:, :])
                        rb = ep.tile([64, 512], f32, tag="recb")
                        nc.gpsimd.partition_broadcast(rb[:, :], rec0[:, :])
                        nc.vector.tensor_mul(
                            ctxh[h][:, q0:q0 + 512], cps[0:64, :], rb[:, :])
                        # this (b,qs) round is destination-slab 4b+qs: ship it
                        # to the a2a buffer as soon as it is normalized
                        nc.sync.dma_start(
                            out=a2a_in[4 * b + qs, 64 * h:64 * (h + 1), :],
                            in_=ctxh[h][:, q0:q0 + 512])

            # ---------- phase 4: AllToAll ----------
            nc.gpsimd.collective_compute(
                "AllToAll", mybir.AluOpType.bypass,
                replica_groups=[list(range(NC))],
                ins=[a2a_in.ap().opt()],
                outs=[a2a_out.ap().opt()])

            # ---------- phase 5: output projection ----------
            _cmB.__exit__(None, None, None)
            _cmA.__exit__(None, None, None)
            _cmO = tc.tile_pool(name="psO", bufs=1, space="PSUM")
            psO = _cmO.__enter__()

            wob = []
            for c in range(NHC):
                t = pp.tile([128, HID], bf16, tag=f"wob{c}", name=f"wob{c}")
                nc.sync.dma_start(out=t[:, :],
                                  in_=woT[128 * c:128 * (c + 1), :])
                wob.append(t)

            # Keep the PE array (HAM) warm across the AllToAll wait: a chain
            # of full-array matmuls anchored on the last ctx tile (via the
            # dumsrc copy) so they cannot run before attention finishes.
            # Result is consumed by a dead-store DMA so DCE keeps the chain.
            dumsrc = pp.tile([128, 512], bf16, tag="dumsrc")
            nc.gpsimd.memset(dumsrc[:, :], 0.0)
            nc.vector.tensor_copy(
                dumsrc[0:64, :], ctxh[1][:, SB - 512:SB])
            dum = psO.tile([128, 512], f32, tag="dum", bufs=1)
            for i in range(230):
                nc.tensor.matmul(
                    dum[:, :], lhsT=wob[0][:, 0:128], rhs=dumsrc[:, :],
                    start=True, stop=True)
            dumr = ep.tile([128, 512], f32, tag="dumr")
            nc.vector.tensor_copy(dumr[:, :], dum[:, :])
            dead = nc.dram_tensor("dead", [128, 512], f32)
            nc.sync.dma_start(out=dead[:, :], in_=dumr[:, :])
            # Load all 8 received o-chunks first (1MB total), then run the
            # accumulation ot-outer so each out-tile finishes early and its
            # bias-add + store overlap the remaining matmuls.
            cxs = []
            for c in range(NHC):
                cx = pp.tile([128, RB], bf16, tag=f"cxb{c}", name=f"cxb{c}")
                nc.sync.dma_start(out=cx[:, :], in_=a2a_out[c, :, :])
                cxs.append(cx)
            for ot in range(8):
                ops = psO.tile([128, 512], f32, tag="ops", bufs=4)
                for c in range(NHC):
                    nc.tensor.matmul(
                        ops[:, :],
                        lhsT=wob[c][:, 128 * ot:128 * (ot + 1)],
                        rhs=cxs[c][:, :],
                        start=(c == 0), stop=(c == NHC - 1))
                osb = ep.tile([128, RB], bf16, tag="osb", bufs=3)
                nc.scalar.activation(
                    osb[:, :], ops[:, :], AF.Identity,
                    bias=bo_sb[:, ot:ot + 1], scale=1.0)
                nc.sync.dma_start(
                    out=out_ext[128 * ot:128 * (ot + 1), :], in_=osb[:, :])
            _cmO.__exit__(None, None, None)

    nc.finalize()
    return nc


def _host_tables():
    inv = 1.0 / (ROPE_BASE ** (np.arange(0, D, 2, dtype=np.float64) / D))
    pos = np.arange(S, dtype=np.float64)
    freqs = np.outer(pos, inv)                      # [S, 32]
    emb = np.concatenate([freqs, freqs], axis=-1)   # [S, 64]
    cosT = np.cos(emb).T.astype(np.float32)         # [64, S]
    sinT = np.sin(emb).T.astype(np.float32)
    sinS = np.concatenate([-sinT[:32], sinT[32:]], axis=0)
    cos2 = np.ascontiguousarray(np.tile(cosT, (2, 2)))   # [128, 2S]
    sin2 = np.ascontiguousarray(np.tile(sinS, (2, 2)))
    return cos2, sin2


def kernel(**inputs):
    import ml_dtypes
    from concourse.bass_utils import run_bass_kernel_spmd

    global _cached, _last_in_maps
    if _cached is None:
        _cached = _build_nc()
    nc = _cached

    bf = ml_dtypes.bfloat16
    hs = np.asarray(inputs["hidden_states"], dtype=np.float32)
    Wq = np.asarray(inputs["Wq"], dtype=np.float32)
    bq = np.asarray(inputs["bq"], dtype=np.float32)
    Wk = np.asarray(inputs["Wk"], dtype=np.float32)
    bk = np.asarray(inputs["bk"], dtype=np.float32)
    Wv = np.asarray(inputs["Wv"], dtype=np.float32)
    bv = np.asarray(inputs["bv"], dtype=np.float32)
    Wo = np.asarray(inputs["Wo"], dtype=np.float32)
    bo = np.asarray(inputs["bo"], dtype=np.float32)

    cos2, sin2 = _host_tables()
    cos2 = cos2.astype(bf)
    sin2 = sin2.astype(bf)
    bo2 = bo + bv @ Wo.T                                 # fold v-bias exactly
    bo2m = np.ascontiguousarray(bo2.reshape(8, 128).T)   # [128, 8]
    xTfull = np.ascontiguousarray(
        np.concatenate([hs[0].T, hs[1].T], axis=1)).astype(bf)  # [1024, 4096]
    woTc = np.ascontiguousarray(Wo.T).astype(bf)

    in_maps = []
    for c in range(NC):
        sl = slice(OSL * c, OSL * (c + 1))
        in_maps.append({
            "xT": xTfull,
            "wqT": np.ascontiguousarray(Wq[sl, :].T).astype(bf),
            "wkT": np.ascontiguousarray(Wk[sl, :].T).astype(bf),
            "wvT": np.ascontiguousarray(Wv[sl, :].T).astype(bf),
            "woT": woTc,
            "bq": np.ascontiguousarray((bq[sl] * 0.125)[:, None]),
            "bk": np.ascontiguousarray(bk[sl][:, None]),
            "bo2": bo2m,
            "cosT": cos2,
            "sinS": sin2,
        })

    _last_in_maps = in_maps
    res = run_bass_kernel_spmd(nc, in_maps, core_ids=list(range(NC)))
    out = np.empty((2, S, HID), dtype=np.float32)
    for c in range(NC):
        b, g = divmod(c, 4)
        out[b, RB * g:RB * (g + 1), :] = res.results[c]["out"].T.astype(np.float32)
    return out


# revision 53
# speedup vs baseline: 1.0297x; 1.0240x over previous
"""AttentionWithRoPE distributed Trainium2 kernel (8 NeuronCores).

Sharding: pure 8-way tensor parallel over heads (2 heads = 128 hidden cols
per core), both batches on every core (seq concatenated to 4096 cols).
Everything stays transposed ([feature, seq] layouts) so no on-device
transposes are needed anywhere:
  - QKV projections consume xT (host-transposed, batch-concat, bf16) as the
    moving operand, streamed in [128, 512] chunks.
  - RoPE applied on qT/kT [d, s] tiles; the 32-row half-rotation is done with
    sbuf->sbuf DMAs (DVE ops must keep operand start-partitions equal), with
    a sign-folded sin table so no negation op is needed.
  - scores^T = kT.T @ qT per (head, batch) in [ks, qs] layout, exp on ScalarE
    (no max subtraction needed: scores are ~N(0,1) here).
  - ctx^T via matmul with a ones-column appended to V (65th column gives the
    softmax denominator for free), normalized via GpSimd partition-broadcast
    of 1/rowsum + one DVE multiply (fuses psum->sbuf copy + cast to bf16).
  - AllToAll (bf16, all 8 cores) exchanges 512-row blocks of ctx^T; received
    slabs are exactly the o-chunks the output projection consumes.
  - Output projection with full Wo produces out^T [1024, 512] for this
    core's 512 global rows; host transposes back (free).
Bias folds (host side): v-bias folds into the output bias exactly (softmax
rows sum to 1); q is pre-scaled by 1/sqrt(64) inside its bias-copy.
Compute dtype bf16 (fp32 PSUM accumulation); all weights/activations are
converted to bf16 on the host.
"""

import numpy as np

HID = 1024
S = 2048
SB = 2 * S       # both batches, seq-concatenated
NHEAD = 16
D = 64
HPC = 2          # heads per core
OSL = 128        # hidden slice per core (HPC * D)
RB = 512         # global row block per core after AllToAll
NC = 8
ROPE_BASE = 10000.0

_cached = None
_last_in_maps = None


def _build_nc():
    import concourse.bacc as bacc
    import concourse.mybir as mybir
    from concourse import tile

    f32 = mybir.dt.float32
    bf16 = mybir.dt.bfloat16
    AF = mybir.ActivationFunctionType

    nc = bacc.Bacc(None, target_bir_lowering=False)

    xT = nc.declare_dram_parameter("xT", [HID, SB], bf16, isOutput=False)
    wqT = nc.declare_dram_parameter("wqT", [HID, OSL], bf16, isOutput=False)
    wkT = nc.declare_dram_parameter("wkT", [HID, OSL], bf16, isOutput=False)
    wvT = nc.declare_dram_parameter("wvT", [HID, OSL], bf16, isOutput=False)
    woT = nc.declare_dram_parameter("woT", [HID, HID], bf16, isOutput=False)
    bqd = nc.declare_dram_parameter("bq", [128, 1], f32, isOutput=False)
    bkd = nc.declare_dram_parameter("bk", [128, 1], f32, isOutput=False)
    bod = nc.declare_dram_parameter("bo2", [128, 8], f32, isOutput=False)
    cosd = nc.declare_dram_parameter("cosT", [128, SB], bf16, isOutput=False)
    sind = nc.declare_dram_parameter("sinS", [128, SB], bf16, isOutput=False)
    out_ext = nc.declare_dram_parameter("out", [HID, RB], bf16, isOutput=True)

    a2a_in = nc.dram_tensor("a2a_in", [NC, OSL, RB], bf16)
    a2a_out = nc.dram_tensor("a2a_out", [NC, OSL, RB], bf16)

    NHC = HID // 128  # 8 hidden chunks
    NSG = SB // 512   # 8 seq groups

    with tile.TileContext(nc) as tc:
        with (
            tc.tile_pool(name="persist", bufs=1) as pp,
            tc.tile_pool(name="xs", bufs=18) as xp,
            tc.tile_pool(name="work", bufs=2) as wp,
            tc.tile_pool(name="exp", bufs=2) as ep,
        ):
            # ---------- consts ----------
            def pload(dram_ap, shape, dt_, tag):
                t = pp.tile(shape, dt_, tag=tag, name=tag)
                nc.sync.dma_start(out=t[:, :], in_=dram_ap)
                return t

            wqb = [pload(wqT[128 * c:128 * (c + 1), :], [128, OSL], bf16,
                         f"wqb{c}") for c in range(NHC)]
            wkb = [pload(wkT[128 * c:128 * (c + 1), :], [128, OSL], bf16,
                         f"wkb{c}") for c in range(NHC)]
            wvb = [pload(wvT[128 * c:128 * (c + 1), :], [128, OSL], bf16,
                         f"wvb{c}") for c in range(NHC)]
            bq_sb = pload(bqd[:, :], [128, 1], f32, "bq")
            bk_sb = pload(bkd[:, :], [128, 1], f32, "bk")
            bo_sb = pload(bod[:, :], [128, 8], f32, "bo")
            # cos/sin are not needed until the first rope quarter (after
            # seq-group 1): load them in column halves so the big transfers
            # don't delay the first projection matmuls.
            cos_sb = pp.tile([128, SB], bf16, tag="cos")
            sin_sb = pp.tile([128, SB], bf16, tag="sin")

            # PSUM pools for phases 1-4 (8 banks exactly); closed before the
            # output projection which needs all 8 banks for its accumulators.
            # Tags are shared across phases: "mm512" serves qk-proj psums then
            # the score tiles; "acc" serves v-proj psums then ctx accumulators.
            _cmA = tc.tile_pool(name="psA", bufs=2, space="PSUM")
            _cmB = tc.tile_pool(name="psB", bufs=4, space="PSUM")
            psA = _cmA.__enter__()
            psB = _cmB.__enter__()

            # ---------- phase 1: QKV projections (x streamed) + fused RoPE ----
            # RoPE for seq-quarter i runs as soon as seq-groups 2i, 2i+1 have
            # projected, overlapping the remaining projection matmuls.
            qsb = wp.tile([128, SB], bf16, tag="qsb", bufs=1)
            ksb = wp.tile([128, SB], bf16, tag="ksb", bufs=1)
            # Per-head rope outputs, d zero-padded to 128 partitions so K=128
            # score matmuls keep the full PE array active (HAM stays warm).
            qrh = [pp.tile([128, SB], bf16, tag=f"qrh{h}", name=f"qrh{h}")
                   for h in range(HPC)]
            krh = [pp.tile([128, SB], bf16, tag=f"krh{h}", name=f"krh{h}")
                   for h in range(HPC)]
            for t in qrh + krh:
                nc.gpsimd.memset(t[64:128, :], 0.0)

            def rope_quarter(src, dsts, q4):
                sl = slice(1024 * q4, 1024 * (q4 + 1))
                qswp = wp.tile([128, 1024], bf16, tag="qswp")
                for blk in range(4):
                    dlo = 32 * blk
                    srow = 32 * (blk + 1) if blk % 2 == 0 else 32 * (blk - 1)
                    nc.sync.dma_start(
                        out=qswp[dlo:dlo + 32, :],
                        in_=src[srow:srow + 32, sl])
                t1 = wp.tile([128, 1024], f32, tag="ropet1")
                t2 = wp.tile([128, 1024], f32, tag="ropet2")
                rt = wp.tile([128, 1024], bf16, tag="ropert")
                nc.vector.tensor_mul(t1[:, :], src[:, sl], cos_sb[:, sl])
                nc.vector.tensor_mul(t2[:, :], qswp[:, :], sin_sb[:, sl])
                nc.vector.tensor_add(rt[:, :], t1[:, :], t2[:, :])
                for h in range(HPC):
                    nc.sync.dma_start(
                        out=dsts[h][0:64, sl],
                        in_=rt[64 * h:64 * (h + 1), :])

            vsb = []
            qkps = {}
            for sg in range(NSG):
                xbt = []
                for c in range(NHC):
                    xb = xp.tile([128, 512], bf16, tag="xb")
                    nc.sync.dma_start(
                        out=xb[:, :],
                        in_=xT[128 * c:128 * (c + 1), 512 * sg:512 * (sg + 1)])
                    xbt.append(xb)
                if sg == 0:
                    # defer the big cos/sin transfers behind the first matmul
                    # so they don't steal DMA bandwidth from the critical
                    # wq/x loads (descriptor-level fair sharing would
                    # otherwise delay the first matmul by ~15us)
                    for qrt in range(4):
                        hs = slice(1024 * qrt, 1024 * (qrt + 1))
                        nc.sync.dma_start(out=cos_sb[:, hs],
                                          in_=cosd[:, hs])
                        nc.sync.dma_start(out=sin_sb[:, hs],
                                          in_=sind[:, hs])
                half = sg % 2
                for key, wb, bias, scale, dest in (
                    ("q", wqb, bq_sb, 0.125, qsb),
                    ("k", wkb, bk_sb, 1.0, ksb),
                ):
                    if half == 0:
                        qkps[key] = psA.tile([128, 1024], f32, tag="mm1024",
                                             name=f"qkps{key}")
                    ps = qkps[key]
                    for c in range(NHC):
                        nc.tensor.matmul(
                            ps[:, 512 * half:512 * (half + 1)],
                            lhsT=wb[c][:, :], rhs=xbt[c][:, :],
                            start=(c == 0), stop=(c == NHC - 1))
                    if half == 1:
                        nc.vector.tensor_scalar(
                            dest[:, 1024 * (sg // 2):1024 * (sg // 2 + 1)],
                            ps[:, :], scale, bias[:, 0:1],
                            mybir.AluOpType.mult, mybir.AluOpType.add)
                for st4 in range(4):
                    st = 4 * sg + st4
                    ps = psB.tile([128, OSL], f32, tag="acc",
                                  padded_shape=[128, 512])
                    for c in range(NHC):
                        nc.tensor.matmul(
                            ps[:, :],
                            lhsT=xbt[c][:, 128 * st4:128 * (st4 + 1)],
                            rhs=wvb[c][:, :],
                            start=(c == 0), stop=(c == NHC - 1))
                    # Per head: [v(64) | ones(1) | zeros(63)] -> M=128 ctx
                    # matmuls keep the full PE array active (HAM stays warm).
                    vt = pp.tile([128, 2 * 128], bf16,
                                 tag=f"vsb{st}", name=f"vsb{st}")
                    nc.gpsimd.memset(vt[:, :], 0.0)
                    for h in range(HPC):
                        nc.gpsimd.memset(vt[:, 128 * h + 64:128 * h + 65], 1.0)
                        nc.vector.tensor_copy(
                            vt[:, 128 * h:128 * h + 64],
                            ps[:, 64 * h:64 * (h + 1)])
                    vsb.append(vt)
                if sg % 2 == 1:
                    rope_quarter(qsb, qrh, sg // 2)
                    rope_quarter(ksb, krh, sg // 2)

            # ---------- phase 3: attention, both heads packed per pass ----------
            # The two heads' K=64 score matmuls are issued back-to-back with
            # lhsT/rhs at base partitions 0 and 64, which auto-derives PE
            # tile_position (0,0)/(64,0): both run concurrently in the array.
            # Per-head ctx tiles so every DVE op starts at partition 0.
            ctxh = [pp.tile([64, SB], bf16, tag=f"ctx{h}", name=f"ctx{h}")
                    for h in range(HPC)]
            for b in range(2):
                for qs in range(4):
                    q0 = S * b + 512 * qs
                    cpsA = psB.tile([128, 512], f32, tag="acc")
                    cpsB = psB.tile([128, 512], f32, tag="acc")
                    for ks in range(16):
                        k0 = S * b + 128 * ks
                        # one 2-bank psum tile holds both heads' score tiles,
                        # so a single wide EXP serves both ctx matmuls
                        sps = psA.tile([128, 1024], f32, tag="mm1024")
                        nc.tensor.matmul(
                            sps[:, 0:512], lhsT=krh[0][:, k0:k0 + 128],
                            rhs=qrh[0][:, q0:q0 + 512],
                            start=True, stop=True)
                        nc.tensor.matmul(
                            sps[:, 512:1024], lhsT=krh[1][:, k0:k0 + 128],
                            rhs=qrh[1][:, q0:q0 + 512],
                            start=True, stop=True)
                        et = ep.tile([128, 1024], bf16, tag="expT", bufs=4)
                        nc.scalar.activation(et[:, :], sps[:, :], AF.Exp)
                        vt = vsb[(S * b) // 128 + ks]
                        nc.tensor.matmul(
                            cpsA[:, :], lhsT=vt[:, 0:128], rhs=et[:, 0:512],
                            start=(ks == 0), stop=(ks == 15))
                        nc.tensor.matmul(
                            cpsB[:, :], lhsT=vt[:, 128:256],
                            rhs=et[:, 512:1024],
                            start=(ks == 0), stop=(ks == 15))
                    for h, cps in ((0, cpsA), (1, cpsB)):
                        # rowsum lives on psum partition 64; hop it to sbuf,
                        # DMA-reshape to [128,4] so reciprocal runs 128 lanes
                        # wide, reshape back, broadcast, normalize.
                        rs65 = ep.tile([65, 512], f32, tag="rec65")
                        nc.vector.tensor_copy(rs65[64:65, :], cps[64:65, :])
                        rsP = ep.tile([128, 4], f32, tag="rsP")
                        nc.sync.dma_start(out=rsP[:, :], in_=rs65[64:65, :])
                        rPr = ep.tile([128, 4], f32, tag="rPr")
                        nc.vector.reciprocal(rPr[:, :], rsP[:, :])
                        rec0 = ep.tile([1, 512], f32, tag="rec0")
                        nc.sync.dma_start(out=rec0[:, :], in_=rPr[:, :])
                        rb = ep.tile([64, 512], f32, tag="recb")
                        nc.gpsimd.partition_broadcast(rb[:, :], rec0[:, :])
                        nc.vector.tensor_mul(
                            ctxh[h][:, q0:q0 + 512], cps[0:64, :], rb[:, :])
                        # this (b,qs) round is destination-slab 4b+qs: ship it
                        # to the a2a buffer as soon as it is normalized
                        nc.sync.dma_start(
                            out=a2a_in[4 * b + qs, 64 * h:64 * (h + 1), :],
                            in_=ctxh[h][:, q0:q0 + 512])

            # ---------- phase 4: AllToAll ----------
            nc.gpsimd.collective_compute(
                "AllToAll", mybir.AluOpType.bypass,
                replica_groups=[list(range(NC))],
                ins=[a2a_in.ap().opt()],
                outs=[a2a_out.ap().opt()])

            # ---------- phase 5: output projection ----------
            _cmB.__exit__(None, None, None)
            _cmA.__exit__(None, None, None)
            _cmO = tc.tile_pool(name="psO", bufs=1, space="PSUM")
            psO = _cmO.__enter__()

            wob = []
            for c in range(NHC):
                t = pp.tile([128, HID], bf16, tag=f"wob{c}", name=f"wob{c}")
                nc.sync.dma_start(out=t[:, :],
                                  in_=woT[128 * c:128 * (c + 1), :])
                wob.append(t)

            # Keep the PE array (HAM) warm across the AllToAll wait: a chain
            # of full-array matmuls anchored on the last ctx tile (via the
            # dumsrc copy) so they cannot run before attention finishes.
            # Result is consumed by a dead-store DMA so DCE keeps the chain.
            dumsrc = pp.tile([128, 512], bf16, tag="dumsrc")
            nc.gpsimd.memset(dumsrc[:, :], 0.0)
            nc.vector.tensor_copy(
                dumsrc[0:64, :], ctxh[1][:, SB - 512:SB])
            dum = psO.tile([128, 512], f32, tag="dum", bufs=1)
            for i in range(230):
                nc.tensor.matmul(
                    dum[:, :], lhsT=wob[0][:, 0:128], rhs=dumsrc[:, :],
                    start=True, stop=True)
            dumr = ep.tile([128, 512], f32, tag="dumr")
            nc.vector.tensor_copy(dumr[:, :], dum[:, :])
            dead = nc.dram_tensor("dead", [128, 512], f32)
            nc.sync.dma_start(out=dead[:, :], in_=dumr[:, :])
            # Load all 8 received o-chunks first (1MB total), then run the
            # accumulation ot-outer so each out-tile finishes early and its
            # bias-add + store overlap the remaining matmuls.
            cxs = []
            for c in range(NHC):
                cx = pp.tile([128, RB], bf16, tag=f"cxb{c}", name=f"cxb{c}")
                nc.sync.dma_start(out=cx[:, :], in_=a2a_out[c, :, :])
                cxs.append(cx)
            for ot in range(8):
                ops = psO.tile([128, 512], f32, tag="ops", bufs=4)
                for c in range(NHC):
                    nc.tensor.matmul(
                        ops[:, :],
                        lhsT=wob[c][:, 128 * ot:128 * (ot + 1)],
                        rhs=cxs[c][:, :],
                        start=(c == 0), stop=(c == NHC - 1))
                osb = ep.tile([128, RB], bf16, tag="osb", bufs=3)
                nc.scalar.activation(
                    osb[:, :], ops[:, :], AF.Identity,
                    bias=bo_sb[:, ot:ot + 1], scale=1.0)
                nc.sync.dma_start(
                    out=out_ext[128 * ot:128 * (ot + 1), :], in_=osb[:, :])
            _cmO.__exit__(None, None, None)

    nc.finalize()
    return nc


def _host_tables():
    inv = 1.0 / (ROPE_BASE ** (np.arange(0, D, 2, dtype=np.float64) / D))
    pos = np.arange(S, dtype=np.float64)
    freqs = np.outer(pos, inv)                      # [S, 32]
    emb = np.concatenate([freqs, freqs], axis=-1)   # [S, 64]
    cosT = np.cos(emb).T.astype(np.float32)         # [64, S]
    sinT = np.sin(emb).T.astype(np.float32)
    sinS = np.concatenate([-sinT[:32], sinT[32:]], axis=0)
    cos2 = np.ascontiguousarray(np.tile(cosT, (2, 2)))   # [128, 2S]
    sin2 = np.ascontiguousarray(np.tile(sinS, (2, 2)))
    return cos2, sin2


def kernel(**inputs):
    import ml_dtypes
    from concourse.bass_utils import run_bass_kernel_spmd

    global _cached, _last_in_maps
    if _cached is None:
        _cached = _build_nc()
    nc = _cached

    bf = ml_dtypes.bfloat16
    hs = np.asarray(inputs["hidden_states"], dtype=np.float32)
    Wq = np.asarray(inputs["Wq"], dtype=np.float32)
    bq = np.asarray(inputs["bq"], dtype=np.float32)
    Wk = np.asarray(inputs["Wk"], dtype=np.float32)
    bk = np.asarray(inputs["bk"], dtype=np.float32)
    Wv = np.asarray(inputs["Wv"], dtype=np.float32)
    bv = np.asarray(inputs["bv"], dtype=np.float32)
    Wo = np.asarray(inputs["Wo"], dtype=np.float32)
    bo = np.asarray(inputs["bo"], dtype=np.float32)

    cos2, sin2 = _host_tables()
    cos2 = cos2.astype(bf)
    sin2 = sin2.astype(bf)
    bo2 = bo + bv @ Wo.T                                 # fold v-bias exactly
    bo2m = np.ascontiguousarray(bo2.reshape(8, 128).T)   # [128, 8]
    xTfull = np.ascontiguousarray(
        np.concatenate([hs[0].T, hs[1].T], axis=1)).astype(bf)  # [1024, 4096]
    woTc = np.ascontiguousarray(Wo.T).astype(bf)

    in_maps = []
    for c in range(NC):
        sl = slice(OSL * c, OSL * (c + 1))
        in_maps.append({
            "xT": xTfull,
            "wqT": np.ascontiguousarray(Wq[sl, :].T).astype(bf),
            "wkT": np.ascontiguousarray(Wk[sl, :].T).astype(bf),
            "wvT": np.ascontiguousarray(Wv[sl, :].T).astype(bf),
            "woT": woTc,
            "bq": np.ascontiguousarray((bq[sl] * 0.125)[:, None]),
            "bk": np.ascontiguousarray(bk[sl][:, None]),
            "bo2": bo2m,
            "cosT": cos2,
            "sinS": sin2,
        })

    _last_in_maps = in_maps
    res = run_bass_kernel_spmd(nc, in_maps, core_ids=list(range(NC)))
    out = np.empty((2, S, HID), dtype=np.float32)
    for c in range(NC):
        b, g = divmod(c, 4)
        out[b, RB * g:RB * (g + 1), :] = res.results[c]["out"].T.astype(np.float32)
    return out
